# revision 4
# baseline (speedup 1.0000x reference)
"""Trainium2 Bass kernel for nn_DistHead (block-diagonal molecule attention).

out = softmax_blockdiag(Q K^T / sqrt(H)) * exp(-invr0 * cdist(Z, Z)) @ V
with Q/K/V = X @ W{q,k,v}^T, block-diagonal over 128 molecules of 64 atoms.

Sharding: 16 whole molecules (1024 rows) per core across 8 cores —
perfectly parallel, zero cross-core communication.

Self-contained: hardcodes shapes from the problem spec; only imports
concourse from /opt/trn_rl_repo.
"""

import sys

if "/opt/trn_rl_repo" not in sys.path:
    sys.path.insert(0, "/opt/trn_rl_repo")

import numpy as np

N, E, H = 8192, 256, 64          # atoms, embedding, head size
NSEG, SEG = 128, 64              # molecules, atoms per molecule
NCORES = 8
RPC = N // NCORES                # rows per core (1024 = 16 molecules)
NT = RPC // 128                  # 128-row tiles per core (2 molecules each)
EC = E // 128                    # embedding chunks of 128

_cache = {}


def _build_nc():
    import concourse.bacc as bacc
    import concourse.tile as tile
    from concourse import mybir
    from concourse.masks import make_identity

    f32 = mybir.dt.float32
    AF = mybir.ActivationFunctionType

    nc = bacc.Bacc(None, target_bir_lowering=False, debug=False)

    xt_d = nc.dram_tensor("xt", [EC, 128, RPC], f32, kind="ExternalInput")
    zat_d = nc.dram_tensor("zat", [5, RPC], f32, kind="ExternalInput")
    zbt_d = nc.dram_tensor("zbt", [5, RPC], f32, kind="ExternalInput")
    wqt_d = nc.dram_tensor("wqt", [EC, 128, H], f32, kind="ExternalInput")
    wkt_d = nc.dram_tensor("wkt", [EC, 128, H], f32, kind="ExternalInput")
    wvt_d = nc.dram_tensor("wvt", [EC, 128, H], f32, kind="ExternalInput")
    ninv_d = nc.dram_tensor("ninv", [1, 1], f32, kind="ExternalInput")
    y_d = nc.dram_tensor("y", [RPC, H], f32, kind="ExternalOutput")

    import concourse.bass as bass

    with tile.TileContext(nc) as tc:
        with (
            tc.tile_pool(name="consts", bufs=1) as consts,
            tc.tile_pool(name="sb", bufs=3) as sb,
            tc.tile_pool(name="sm", bufs=4) as sm,
            tc.tile_pool(name="ps", bufs=6, space="PSUM") as ps,
            tc.tile_pool(name="ps2", bufs=2, space="PSUM") as ps2,
        ):
            ident = consts.tile([128, 128], f32, tag="ident")
            make_identity(nc, ident)

            ninv = consts.tile([128, 1], f32, tag="ninv")
            src = ninv_d[:, :]
            nc.sync.dma_start(
                out=ninv,
                in_=bass.AP(tensor=src.tensor, offset=src.offset, ap=[[0, 128], [1, 1]]),
            )

            w_sb = {}
            for nm, dram in (("q", wqt_d), ("k", wkt_d), ("v", wvt_d)):
                for c in range(EC):
                    t = consts.tile([128, H], f32, tag=f"w{nm}{c}")
                    nc.sync.dma_start(out=t, in_=dram[c])
                    w_sb[nm, c] = t

            xt = []
            for c in range(EC):
                t = consts.tile([128, RPC], f32, tag=f"xt{c}")
                nc.sync.dma_start(out=t, in_=xt_d[c])
                xt.append(t)

            zat = consts.tile([5, RPC], f32, tag="zat")
            nc.sync.dma_start(out=zat, in_=zat_d[:, :])
            zbt = consts.tile([5, RPC], f32, tag="zbt")
            nc.sync.dma_start(out=zbt, in_=zbt_d[:, :])

            # Q^T, K^T: [H, RPC], contraction over embedding in 2 chunks.
            qt = consts.tile([H, RPC], f32, tag="qt")
            kt = consts.tile([H, RPC], f32, tag="kt")
            for nm, dst in (("q", qt), ("k", kt)):
                for h in range(RPC // 512):
                    p = ps.tile([H, 512], f32, tag="mm")
                    for c in range(EC):
                        nc.tensor.matmul(
                            p,
                            lhsT=w_sb[nm, c],
                            rhs=xt[c][:, h * 512 : (h + 1) * 512],
                            start=(c == 0),
                            stop=(c == EC - 1),
                        )
                    nc.scalar.copy(out=dst[:, h * 512 : (h + 1) * 512], in_=p)

            # V row-major, one 64-row slot per molecule so every PV matmul
            # reads keys from base partition 0.
            nmol = RPC // SEG
            v_sb = consts.tile([SEG, nmol, H], f32, tag="v")
            for m in range(nmol):
                p = ps.tile([SEG, H], f32, tag="mm")
                for c in range(EC):
                    nc.tensor.matmul(
                        p,
                        lhsT=xt[c][:, m * SEG : (m + 1) * SEG],
                        rhs=w_sb["v", c],
                        start=(c == 0),
                        stop=(c == EC - 1),
                    )
                nc.scalar.copy(out=v_sb[:, m, :], in_=p)

            # Main loop: one 128-row tile = two molecules; each row only ever
            # scores against its own molecule's 64 keys, so no mask is needed.
            for t in range(NT):
                ra = slice(t * 128, t * 128 + SEG)        # molecule a columns
                rb = slice(t * 128 + SEG, (t + 1) * 128)  # molecule b columns

                s_ps = ps.tile([128, SEG], f32, tag="mm")
                nc.tensor.matmul(s_ps[0:SEG], lhsT=qt[:, ra], rhs=kt[:, ra], start=True, stop=True)
                nc.tensor.matmul(s_ps[SEG:128], lhsT=qt[:, rb], rhs=kt[:, rb], start=True, stop=True)

                d_ps = ps.tile([128, SEG], f32, tag="mm")
                nc.tensor.matmul(d_ps[0:SEG], lhsT=zat[:, ra], rhs=zbt[:, ra], start=True, stop=True)
                nc.tensor.matmul(d_ps[SEG:128], lhsT=zat[:, rb], rhs=zbt[:, rb], start=True, stop=True)

                dist = sb.tile([128, SEG], f32, tag="dist")
                nc.vector.tensor_scalar_max(out=dist, in0=d_ps, scalar1=0.0)
                nc.scalar.activation(out=dist, in_=dist, func=AF.Sqrt)
                expd = sb.tile([128, SEG], f32, tag="expd")
                nc.scalar.activation(out=expd, in_=dist, func=AF.Exp, scale=ninv[:, :])

                # exp(S) with fused row-sum; |S| <= ||q||*||k||/8 ~ 8, no
                # max-subtraction needed for fp32 range.
                e = sb.tile([128, SEG], f32, tag="e")
                rowsum = sm.tile([128, 1], f32, tag="rowsum")
                nc.scalar.activation(out=e, in_=s_ps, func=AF.Exp, accum_out=rowsum)
                rinv = sm.tile([128, 1], f32, tag="rinv")
                nc.vector.reciprocal(out=rinv, in_=rowsum)

                wei = sb.tile([128, SEG], f32, tag="wei")
                nc.vector.tensor_mul(out=wei, in0=e, in1=expd)

                # wei^T [SEG, 128] in one PE transpose; both molecule halves
                # land at base partition 0 in the free dim.
                wt_ps = ps2.tile([SEG, 128], f32, tag="wt")
                nc.tensor.transpose(wt_ps, wei, ident)
                wt_sb = sb.tile([SEG, 128], f32, tag="wt_sb")
                nc.vector.tensor_copy(out=wt_sb, in_=wt_ps)

                o_ps = ps.tile([128, H], f32, tag="mm")
                nc.tensor.matmul(o_ps[0:SEG], lhsT=wt_sb[:, 0:SEG], rhs=v_sb[:, 2 * t, :], start=True, stop=True)
                nc.tensor.matmul(o_ps[SEG:128], lhsT=wt_sb[:, SEG:128], rhs=v_sb[:, 2 * t + 1, :], start=True, stop=True)

                o_sb = sb.tile([128, H], f32, tag="o")
                nc.vector.tensor_scalar_mul(out=o_sb, in0=o_ps, scalar1=rinv)
                nc.sync.dma_start(out=y_d[t * 128 : (t + 1) * 128, :], in_=o_sb)

    nc.compile()
    return nc


def _get_nc():
    if "nc" not in _cache:
        _cache["nc"] = _build_nc()
    return _cache["nc"]


def _prepare_in_maps(X, Z, Wk, Wq, Wv, invr0):
    X = np.ascontiguousarray(X, dtype=np.float32)
    Z = np.ascontiguousarray(Z, dtype=np.float32)
    xt_full = np.ascontiguousarray(X.T).reshape(EC, 128, N)
    z2 = np.sum(Z * Z, axis=-1)
    ones = np.ones(N, dtype=np.float32)
    zt = np.ascontiguousarray(Z.T)
    zat_full = np.concatenate([z2[None], ones[None], -2.0 * zt], axis=0).astype(np.float32)
    zbt_full = np.concatenate([ones[None], z2[None], zt], axis=0).astype(np.float32)

    scale = np.float32(H) ** np.float32(-0.5)
    wqt = np.ascontiguousarray((Wq.T * scale).astype(np.float32)).reshape(EC, 128, H)
    wkt = np.ascontiguousarray(Wk.T.astype(np.float32)).reshape(EC, 128, H)
    wvt = np.ascontiguousarray(Wv.T.astype(np.float32)).reshape(EC, 128, H)
    ninv = np.array([[-float(invr0.reshape(-1)[0])]], dtype=np.float32)

    in_maps = []
    for d in range(NCORES):
        s, e = d * RPC, (d + 1) * RPC
        in_maps.append(
            {
                "xt": np.ascontiguousarray(xt_full[:, :, s:e]),
                "zat": np.ascontiguousarray(zat_full[:, s:e]),
                "zbt": np.ascontiguousarray(zbt_full[:, s:e]),
                "wqt": wqt,
                "wkt": wkt,
                "wvt": wvt,
                "ninv": ninv,
            }
        )
    return in_maps


def _run(in_maps, trace=False, **kwargs):
    from concourse.bass_utils import run_bass_kernel_spmd

    nc = _get_nc()
    return run_bass_kernel_spmd(nc, in_maps, list(range(NCORES)), trace=trace, **kwargs)


def _numpy_fallback(X, Z, Wk, Wq, Wv, invr0, ptr):
    """Reference-exact fallback for ptr layouts other than 128 x 64."""
    X = np.asarray(X, dtype=np.float32)
    Z = np.asarray(Z, dtype=np.float32)
    n = X.shape[0]
    K = X @ Wk.T
    Q = X @ Wq.T
    V = X @ Wv.T
    seg = np.searchsorted(np.asarray(ptr)[1:], np.arange(n), side="right")
    out = np.zeros((n, Wk.shape[0]), dtype=np.float32)
    inv = float(np.asarray(invr0).reshape(-1)[0])
    hs = Wk.shape[0] ** -0.5
    for s in np.unique(seg):
        idx = np.nonzero(seg == s)[0]
        q, k, v, z = Q[idx], K[idx], V[idx], Z[idx]
        wei = (q @ k.T) * hs
        wei = wei - wei.max(axis=-1, keepdims=True)
        wei = np.exp(wei)
        wei /= wei.sum(axis=-1, keepdims=True)
        d2 = np.maximum(
            (z * z).sum(-1)[:, None] + (z * z).sum(-1)[None, :] - 2.0 * (z @ z.T), 0.0
        )
        dist = np.sqrt(np.where(d2 > 0, d2, 1.0)) * (d2 > 0)
        wei = wei * np.exp(-inv * dist)
        out[idx] = wei @ v
    return out


def kernel(X, Z, Wk, Wq, Wv, invr0, ptr):
    ptr = np.asarray(ptr)
    if not (
        X.shape == (N, E)
        and Wk.shape == (H, E)
        and ptr.shape == (NSEG + 1,)
        and np.array_equal(ptr, np.arange(NSEG + 1, dtype=ptr.dtype) * SEG)
    ):
        return _numpy_fallback(X, Z, Wk, Wq, Wv, invr0, ptr)

    in_maps = _prepare_in_maps(X, Z, Wk, Wq, Wv, invr0)
    res = _run(in_maps, trace=False)
    out = np.empty((N, H), dtype=np.float32)
    for d in range(NCORES):
        out[d * RPC : (d + 1) * RPC] = res.results[d]["y"]
    return out


# revision 5
# speedup vs baseline: 1.3208x; 1.3208x over previous
"""Trainium2 Bass kernel for nn_DistHead (block-diagonal molecule attention).

out = softmax_blockdiag(Q K^T / sqrt(H)) * exp(-invr0 * cdist(Z, Z)) @ V
with Q/K/V = X @ W{q,k,v}^T, block-diagonal over 128 molecules of 64 atoms.

Sharding: 16 whole molecules (1024 rows) per core across 8 cores —
perfectly parallel, zero cross-core communication.

Key tricks:
- Block-diagonal mask folded into the score matmul: Q^T/K^T get two extra
  contraction rows (+-16 patterns) that add exactly 0 for same-molecule
  pairs and -512 for cross-molecule pairs inside a 128-row tile, so
  exp() underflows off-block scores to exactly 0. No mask ops at all.
- Pairwise distance^2 via one K=5 matmul using augmented coordinates
  [z2, 1, -2z] x [1, z2, z].
- All elementwise work batched into single [128, 1024]-wide ops.

Self-contained: hardcodes shapes from the problem spec; only imports
concourse from /opt/trn_rl_repo.
"""

import sys

if "/opt/trn_rl_repo" not in sys.path:
    sys.path.insert(0, "/opt/trn_rl_repo")

import numpy as np

N, E, H = 8192, 256, 64          # atoms, embedding, head size
NSEG, SEG = 128, 64              # molecules, atoms per molecule
NCORES = 8
RPC = N // NCORES                # rows per core (1024 = 16 molecules)
NT = RPC // 128                  # 128-row tiles per core (2 molecules each)
EC = E // 128                    # embedding chunks of 128

_cache = {}


def _build_nc():
    import concourse.bacc as bacc
    import concourse.bass as bass
    import concourse.tile as tile
    from concourse import mybir
    from concourse.masks import make_identity

    f32 = mybir.dt.float32
    AF = mybir.ActivationFunctionType
    X_AX = mybir.AxisListType.X

    nc = bacc.Bacc(None, target_bir_lowering=False, debug=False)

    xt_d = nc.dram_tensor("xt", [EC, 128, RPC], f32, kind="ExternalInput")
    zat_d = nc.dram_tensor("zat", [5, RPC], f32, kind="ExternalInput")
    zbt_d = nc.dram_tensor("zbt", [5, RPC], f32, kind="ExternalInput")
    wqt_d = nc.dram_tensor("wqt", [EC, 128, H], f32, kind="ExternalInput")
    wkt_d = nc.dram_tensor("wkt", [EC, 128, H], f32, kind="ExternalInput")
    wvt_d = nc.dram_tensor("wvt", [EC, 128, H], f32, kind="ExternalInput")
    qaug_d = nc.dram_tensor("qaug", [2, RPC], f32, kind="ExternalInput")
    kaug_d = nc.dram_tensor("kaug", [2, RPC], f32, kind="ExternalInput")
    ninv_d = nc.dram_tensor("ninv", [1, 1], f32, kind="ExternalInput")
    y_d = nc.dram_tensor("y", [RPC, H], f32, kind="ExternalOutput")

    with tile.TileContext(nc) as tc:
        with (
            tc.tile_pool(name="consts", bufs=1) as consts,
            tc.tile_pool(name="sb", bufs=3) as sb,
            tc.tile_pool(name="sm", bufs=2) as sm,
            tc.tile_pool(name="wide", bufs=1) as wide,
            tc.tile_pool(name="psm", bufs=4, space="PSUM") as psm,
            tc.tile_pool(name="psb", bufs=1, space="PSUM") as psb,
        ):
            ident = consts.tile([128, 128], f32, tag="ident")
            make_identity(nc, ident)

            ninv = consts.tile([128, 1], f32, tag="ninv")
            src = ninv_d[:, :]
            nc.sync.dma_start(
                out=ninv,
                in_=bass.AP(tensor=src.tensor, offset=src.offset, ap=[[0, 128], [1, 1]]),
            )

            w_sb = {}
            for nm, dram in (("q", wqt_d), ("k", wkt_d), ("v", wvt_d)):
                for c in range(EC):
                    t = consts.tile([128, H], f32, tag=f"w{nm}{c}")
                    nc.sync.dma_start(out=t, in_=dram[c])
                    w_sb[nm, c] = t

            xt = []
            for c in range(EC):
                t = consts.tile([128, RPC], f32, tag=f"xt{c}")
                nc.sync.dma_start(out=t, in_=xt_d[c])
                xt.append(t)

            zat = consts.tile([5, RPC], f32, tag="zat")
            nc.sync.dma_start(out=zat, in_=zat_d[:, :])
            zbt = consts.tile([5, RPC], f32, tag="zbt")
            nc.sync.dma_start(out=zbt, in_=zbt_d[:, :])

            # Q^T / K^T with two augmented mask rows each: [66, RPC].
            qt = consts.tile([H + 2, RPC], f32, tag="qt")
            kt = consts.tile([H + 2, RPC], f32, tag="kt")
            nc.sync.dma_start(out=qt[H : H + 2, :], in_=qaug_d[:, :])
            nc.sync.dma_start(out=kt[H : H + 2, :], in_=kaug_d[:, :])
            for nm, dst in (("q", qt), ("k", kt)):
                for h in range(RPC // 512):
                    p = psm.tile([H, 512], f32, tag="mi")
                    for c in range(EC):
                        nc.tensor.matmul(
                            p,
                            lhsT=w_sb[nm, c],
                            rhs=xt[c][:, h * 512 : (h + 1) * 512],
                            start=(c == 0),
                            stop=(c == EC - 1),
                        )
                    nc.scalar.copy(out=dst[0:H, h * 512 : (h + 1) * 512], in_=p)

            # V row-major: [128, NT, H]; partition p of slot t is row t*128+p.
            v_sb = consts.tile([128, NT, H], f32, tag="v")
            for t in range(NT):
                p = psm.tile([128, H], f32, tag="mi")
                for c in range(EC):
                    nc.tensor.matmul(
                        p,
                        lhsT=xt[c][:, t * 128 : (t + 1) * 128],
                        rhs=w_sb["v", c],
                        start=(c == 0),
                        stop=(c == EC - 1),
                    )
                nc.scalar.copy(out=v_sb[:, t, :], in_=p)

            # Scores and distance^2 for all NT tiles into two 2-bank psum
            # tensors; each matmul's 128-col slice stays inside one bank.
            s_ps = psb.tile([128, NT, 128], f32, tag="s")
            d_ps = psb.tile([128, NT, 128], f32, tag="d")
            for t in range(NT):
                rt = slice(t * 128, (t + 1) * 128)
                nc.tensor.matmul(s_ps[:, t, :], lhsT=qt[:, rt], rhs=kt[:, rt], start=True, stop=True)
                nc.tensor.matmul(d_ps[:, t, :], lhsT=zat[:, rt], rhs=zbt[:, rt], start=True, stop=True)

            # exp(S): off-block entries are ~-504 -> exactly 0, so row sums
            # over the dense 128 columns are already correct.
            e = wide.tile([128, NT, 128], f32, tag="e")
            nc.scalar.activation(out=e, in_=s_ps, func=AF.Exp)
            rowsum = sm.tile([128, NT], f32, tag="rowsum")
            nc.vector.reduce_sum(out=rowsum, in_=e, axis=X_AX)
            rinv = sm.tile([128, NT], f32, tag="rinv")
            nc.vector.reciprocal(out=rinv, in_=rowsum)

            # exp(-invr0 * sqrt(max(d2, 0))), then wei = e * decay (in place).
            dist = wide.tile([128, NT, 128], f32, tag="dist")
            nc.vector.tensor_scalar_max(out=dist, in0=d_ps, scalar1=0.0)
            nc.scalar.activation(out=dist, in_=dist, func=AF.Sqrt)
            nc.scalar.activation(out=dist, in_=dist, func=AF.Exp, scale=ninv[:, :])
            nc.vector.tensor_mul(out=e, in0=e, in1=dist)

            # Per tile: transpose wei, dense PV (off-block wei == 0), scale
            # rows by 1/rowsum post-matmul, store.
            for t in range(NT):
                wt_ps = psm.tile([128, 128], f32, tag="mi")
                nc.tensor.transpose(wt_ps, e[:, t, :], ident)
                wt_sb = sb.tile([128, 128], f32, tag="wt")
                nc.vector.tensor_copy(out=wt_sb, in_=wt_ps)

                o_ps = psm.tile([128, H], f32, tag="mi")
                nc.tensor.matmul(o_ps, lhsT=wt_sb, rhs=v_sb[:, t, :], start=True, stop=True)
                o_sb = sb.tile([128, H], f32, tag="o")
                nc.vector.tensor_scalar_mul(out=o_sb, in0=o_ps, scalar1=rinv[:, t : t + 1])
                nc.sync.dma_start(out=y_d[t * 128 : (t + 1) * 128, :], in_=o_sb)

    nc.compile()
    return nc


def _get_nc():
    if "nc" not in _cache:
        _cache["nc"] = _build_nc()
    return _cache["nc"]


def _prepare_in_maps(X, Z, Wk, Wq, Wv, invr0):
    X = np.ascontiguousarray(X, dtype=np.float32)
    Z = np.ascontiguousarray(Z, dtype=np.float32)
    xt_full = np.ascontiguousarray(X.T).reshape(EC, 128, N)
    z2 = np.sum(Z * Z, axis=-1)
    ones = np.ones(N, dtype=np.float32)
    zt = np.ascontiguousarray(Z.T)
    zat_full = np.concatenate([z2[None], ones[None], -2.0 * zt], axis=0).astype(np.float32)
    zbt_full = np.concatenate([ones[None], z2[None], zt], axis=0).astype(np.float32)

    scale = np.float32(H) ** np.float32(-0.5)
    wqt = np.ascontiguousarray((Wq.T * scale).astype(np.float32)).reshape(EC, 128, H)
    wkt = np.ascontiguousarray(Wk.T.astype(np.float32)).reshape(EC, 128, H)
    wvt = np.ascontiguousarray(Wv.T.astype(np.float32)).reshape(EC, 128, H)
    ninv = np.array([[-float(np.asarray(invr0).reshape(-1)[0])]], dtype=np.float32)

    # Mask rows: same-molecule pairs within a 128-row tile add exactly 0,
    # cross-molecule pairs add -512 (256 and +-16 are exact in fp32).
    sig = np.where((np.arange(RPC) % 128) < SEG, 16.0, -16.0).astype(np.float32)
    ones_r = np.ones(RPC, dtype=np.float32)
    qaug = np.stack([ones_r, sig]).astype(np.float32)
    kaug = np.stack([-256.0 * ones_r, sig]).astype(np.float32)

    in_maps = []
    for d in range(NCORES):
        s, e = d * RPC, (d + 1) * RPC
        in_maps.append(
            {
                "xt": np.ascontiguousarray(xt_full[:, :, s:e]),
                "zat": np.ascontiguousarray(zat_full[:, s:e]),
                "zbt": np.ascontiguousarray(zbt_full[:, s:e]),
                "wqt": wqt,
                "wkt": wkt,
                "wvt": wvt,
                "qaug": qaug,
                "kaug": kaug,
                "ninv": ninv,
            }
        )
    return in_maps


def _run(in_maps, trace=False, **kwargs):
    from concourse.bass_utils import run_bass_kernel_spmd

    nc = _get_nc()
    return run_bass_kernel_spmd(nc, in_maps, list(range(NCORES)), trace=trace, **kwargs)


def _numpy_fallback(X, Z, Wk, Wq, Wv, invr0, ptr):
    """Reference-exact fallback for ptr layouts other than 128 x 64."""
    X = np.asarray(X, dtype=np.float32)
    Z = np.asarray(Z, dtype=np.float32)
    n = X.shape[0]
    K = X @ Wk.T
    Q = X @ Wq.T
    V = X @ Wv.T
    seg = np.searchsorted(np.asarray(ptr)[1:], np.arange(n), side="right")
    out = np.zeros((n, Wk.shape[0]), dtype=np.float32)
    inv = float(np.asarray(invr0).reshape(-1)[0])
    hs = Wk.shape[0] ** -0.5
    for s in np.unique(seg):
        idx = np.nonzero(seg == s)[0]
        q, k, v, z = Q[idx], K[idx], V[idx], Z[idx]
        wei = (q @ k.T) * hs
        wei = wei - wei.max(axis=-1, keepdims=True)
        wei = np.exp(wei)
        wei /= wei.sum(axis=-1, keepdims=True)
        d2 = np.maximum(
            (z * z).sum(-1)[:, None] + (z * z).sum(-1)[None, :] - 2.0 * (z @ z.T), 0.0
        )
        dist = np.sqrt(np.where(d2 > 0, d2, 1.0)) * (d2 > 0)
        wei = wei * np.exp(-inv * dist)
        out[idx] = wei @ v
    return out


def kernel(X, Z, Wk, Wq, Wv, invr0, ptr):
    ptr = np.asarray(ptr)
    if not (
        X.shape == (N, E)
        and Wk.shape == (H, E)
        and ptr.shape == (NSEG + 1,)
        and np.array_equal(ptr, np.arange(NSEG + 1, dtype=ptr.dtype) * SEG)
    ):
        return _numpy_fallback(X, Z, Wk, Wq, Wv, invr0, ptr)

    in_maps = _prepare_in_maps(X, Z, Wk, Wq, Wv, invr0)
    res = _run(in_maps, trace=False)
    out = np.empty((N, H), dtype=np.float32)
    for d in range(NCORES):
        out[d * RPC : (d + 1) * RPC] = res.results[d]["y"]
    return out
